# revision 2
# baseline (speedup 1.0000x reference)
"""Trainium2 Bass kernel for nn_LocalAttention_28518582845970.

Observation: the reference projects the full 256x256x1024 grid through
Q/K/V/O, but returns only out[px, py] -- a single 1024-vector.  That
vector depends on one window row: 129 tokens of K/V (grid row rows[px],
cols), one Q token, and (by linearity + softmax shift-invariance +
sum(attn)==1) four 1024x1024 weight matrices.

Sharding: split the 1024 projection features across the 8 cores (128
each).  Each core computes its K/Q feature slice, partial attention
scores (contracted over its features), AllReduce(129 floats) the scores,
redundantly softmaxes, applies attention to its V-feature slice,
AllGathers the 1024-dim context, and produces its 128 output features.
Host concatenates the 8x128 outputs.

Per-core traffic: ~2.5 MiB of weight slices instead of 16.8 MiB full
weights; two sub-KB collectives.
"""

import os
import sys

os.environ.setdefault("JAX_PLATFORMS", "axon")

for _p in ("/opt/trn_rl_repo", "/root/.axon_site/_ro/trn_rl_repo"):
    if os.path.isdir(_p) and _p not in sys.path:
        sys.path.append(_p)

import numpy as np

import concourse.bass as bass
import concourse.mybir as mybir
import concourse.tile as tile
from concourse import bacc
from concourse.bass_utils import run_bass_kernel_spmd
from concourse.masks import make_identity

N_CORES = 8
E = 1024
EC = E // 128  # 8 contraction chunks of 128
WIN = 64
H = W = 256
SCALE = 1.0 / 32.0  # 1/sqrt(1024), exact in fp32
F32 = mybir.dt.float32

_BUILD_CACHE: dict = {}


def _build(L: int, qidx: int):
    """Build the per-core Bass program (identical on all cores).

    L    -- window length (tokens in the attention row), 129 normally.
    qidx -- index of the query token within the window cols.
    """
    KA = min(128, L)
    KB = L - KA  # leftover tokens beyond the first 128 (1 normally)

    nc = bacc.Bacc(None, target_bir_lowering=False, debug=False)

    tokT_d = nc.dram_tensor("tokT", [E, L], F32, kind="ExternalInput")
    wqT_d = nc.dram_tensor("wqT", [E, 128], F32, kind="ExternalInput")
    wkT_d = nc.dram_tensor("wkT", [E, 128], F32, kind="ExternalInput")
    wvT_d = nc.dram_tensor("wvT", [E, 128], F32, kind="ExternalInput")
    woT_d = nc.dram_tensor("woT", [E, 128], F32, kind="ExternalInput")
    bias_d = nc.dram_tensor("biases", [128, 4], F32, kind="ExternalInput")
    out_d = nc.dram_tensor("out", [128], F32, kind="ExternalOutput")

    with tile.TileContext(nc) as tc:
        with (
            tc.tile_pool(name="consts", bufs=1) as consts,
            tc.tile_pool(name="sbw", bufs=1) as sbw,
            tc.tile_pool(name="psA", bufs=2, space="PSUM") as psA,
            tc.tile_pool(name="psS", bufs=4, space="PSUM") as psS,
            tc.tile_pool(name="dram", bufs=1, space="DRAM") as dr,
        ):
            # ---- loads ----
            wq_sb = consts.tile([128, EC, 128], F32)
            nc.sync.dma_start(out=wq_sb, in_=wqT_d.rearrange("(c p) f -> p c f", p=128))
            wk_sb = consts.tile([128, EC, 128], F32)
            nc.sync.dma_start(out=wk_sb, in_=wkT_d.rearrange("(c p) f -> p c f", p=128))
            wv_sb = consts.tile([128, EC, 128], F32)
            nc.sync.dma_start(out=wv_sb, in_=wvT_d.rearrange("(c p) f -> p c f", p=128))
            wo_sb = consts.tile([128, EC, 128], F32)
            nc.sync.dma_start(out=wo_sb, in_=woT_d.rearrange("(c p) f -> p c f", p=128))
            tok_sb = consts.tile([128, EC, L], F32)
            nc.sync.dma_start(out=tok_sb, in_=tokT_d.rearrange("(c p) k -> p c k", p=128))
            bias_sb = consts.tile([128, 4], F32)
            nc.sync.dma_start(out=bias_sb, in_=bias_d[:, :])
            ident = consts.tile([128, 128], F32)
            make_identity(nc, ident)

            # ---- K projection: K[f, k] = Wk[fc] @ tokens.T (+bk) ----
            k_ps = psA.tile([128, L], F32, tag="big")
            for c in range(EC):
                nc.tensor.matmul(
                    k_ps, wk_sb[:, c, :], tok_sb[:, c, :],
                    start=(c == 0), stop=(c == EC - 1),
                )
            k_sb = sbw.tile([128, L], F32)
            nc.vector.tensor_scalar_add(k_sb, k_ps, bias_sb[:, 1:2])

            # ---- q projection: q[f] = Wq[fc] @ t_q (+bq) ----
            q_ps = psS.tile([128, 1], F32, tag="s")
            for c in range(EC):
                nc.tensor.matmul(
                    q_ps, wq_sb[:, c, :], tok_sb[:, c, qidx:qidx + 1],
                    start=(c == 0), stop=(c == EC - 1),
                )
            q_sb = sbw.tile([128, 1], F32)
            nc.vector.tensor_scalar_add(q_sb, q_ps, bias_sb[:, 0:1])

            # ---- partial scores s_c[k] = K[:,k] . q  (contract over f) ----
            sA_ps = psS.tile([128, 1], F32, tag="s")
            nc.tensor.matmul(sA_ps, k_sb[:, 0:KA], q_sb, start=True, stop=True)
            if KB:
                sB_ps = psS.tile([KB, 1], F32, tag="s")
                nc.tensor.matmul(sB_ps, k_sb[:, KA:L], q_sb, start=True, stop=True)

            sA_sb = sbw.tile([KA, 1], F32)
            nc.vector.tensor_copy(sA_sb, sA_ps)
            if KB:
                sB_sb = sbw.tile([KB, 1], F32)
                nc.vector.tensor_copy(sB_sb, sB_ps)
            cc_in = dr.tile([L], F32)
            cc_in2 = cc_in.rearrange("(k o) -> k o", o=1)
            nc.sync.dma_start(out=cc_in2[0:KA], in_=sA_sb)
            if KB:
                nc.sync.dma_start(out=cc_in2[KA:L], in_=sB_sb)
            cc_out = dr.tile([L], F32, addr_space="Shared")
            nc.gpsimd.collective_compute(
                "AllReduce", mybir.AluOpType.add,
                replica_groups=[list(range(N_CORES))],
                ins=[cc_in[:]], outs=[cc_out[:]],
            )

            # ---- softmax over L scores (redundant on every core) ----
            sc_row = sbw.tile([1, L], F32)
            nc.sync.dma_start(out=sc_row, in_=cc_out.rearrange("(o k) -> o k", o=1))
            mx = sbw.tile([1, 1], F32)
            nc.vector.tensor_reduce(mx, sc_row, axis=mybir.AxisListType.X,
                                    op=mybir.AluOpType.max)
            nmx = sbw.tile([1, 1], F32)
            nc.scalar.mul(nmx, mx, -SCALE)
            ex_row = sbw.tile([1, L], F32)
            nc.scalar.activation(ex_row, sc_row, mybir.ActivationFunctionType.Exp,
                                 bias=nmx, scale=SCALE)
            sm = sbw.tile([1, 1], F32)
            nc.vector.tensor_reduce(sm, ex_row, axis=mybir.AxisListType.X,
                                    op=mybir.AluOpType.add)
            rs = sbw.tile([1, 1], F32)
            nc.vector.reciprocal(rs, sm)
            at_row = sbw.tile([1, L], F32)
            nc.vector.tensor_scalar_mul(at_row, ex_row, rs)

            # transpose attn -> column layout [KA,1] (+ tail [KB,1])
            atT_ps = psS.tile([128, 1], F32, tag="s")
            nc.tensor.transpose(atT_ps, at_row[0:1, 0:KA], ident[0:1, 0:1])
            at_colA = sbw.tile([128, 1], F32)
            nc.vector.tensor_copy(at_colA, atT_ps)
            if KB:
                at_tail = sbw.tile([KB, 1], F32)
                nc.vector.tensor_copy(at_tail, at_row[0:1, KA:L])

            # ---- V in [k, f] layout: V = tokens @ Wv[fc].T ----
            vA_ps = psA.tile([KA, 128], F32, tag="big")
            for c in range(EC):
                nc.tensor.matmul(
                    vA_ps, tok_sb[:, c, 0:KA], wv_sb[:, c, :],
                    start=(c == 0), stop=(c == EC - 1),
                )
            vA_sb = sbw.tile([KA, 128], F32)
            nc.vector.tensor_copy(vA_sb, vA_ps)
            if KB:
                vB_ps = psA.tile([KB, 128], F32, tag="big")
                for c in range(EC):
                    nc.tensor.matmul(
                        vB_ps, tok_sb[:, c, KA:L], wv_sb[:, c, :],
                        start=(c == 0), stop=(c == EC - 1),
                    )
                vB_sb = sbw.tile([KB, 128], F32)
                nc.vector.tensor_copy(vB_sb, vB_ps)

            # ---- ctx_c = V.T @ attn (+bv) ----
            ctx_ps = psS.tile([128, 1], F32, tag="s")
            nc.tensor.matmul(ctx_ps, vA_sb, at_colA, start=True, stop=(KB == 0))
            if KB:
                nc.tensor.matmul(ctx_ps, vB_sb, at_tail, start=False, stop=True)
            ctx_sb = sbw.tile([128, 1], F32)
            nc.vector.tensor_scalar_add(ctx_sb, ctx_ps, bias_sb[:, 2:3])

            # ---- AllGather full ctx [1024] ----
            ag_in = dr.tile([128], F32)
            nc.sync.dma_start(out=ag_in.rearrange("(p o) -> p o", o=1), in_=ctx_sb)
            ag_out = dr.tile([E], F32, addr_space="Shared")
            nc.gpsimd.collective_compute(
                "AllGather", mybir.AluOpType.bypass,
                replica_groups=[list(range(N_CORES))],
                ins=[ag_in[:]], outs=[ag_out[:]],
            )
            ctx_row = sbw.tile([1, E], F32)
            nc.sync.dma_start(out=ctx_row, in_=ag_out.rearrange("(o k) -> o k", o=1))

            # transpose ctx into [128, EC] column layout
            ctxT_sb = sbw.tile([128, EC], F32)
            for c in range(EC):
                cT_ps = psS.tile([128, 1], F32, tag="s", name=f"cT{c}")
                nc.tensor.transpose(
                    cT_ps, ctx_row[0:1, 128 * c:128 * (c + 1)], ident[0:1, 0:1]
                )
                nc.vector.tensor_copy(ctxT_sb[:, c:c + 1], cT_ps)

            # ---- out_c = Wo[fc] @ ctx (+bo) ----
            o_ps = psS.tile([128, 1], F32, tag="s")
            for c in range(EC):
                nc.tensor.matmul(
                    o_ps, wo_sb[:, c, :], ctxT_sb[:, c:c + 1],
                    start=(c == 0), stop=(c == EC - 1),
                )
            o_sb = sbw.tile([128, 1], F32)
            nc.vector.tensor_scalar_add(o_sb, o_ps, bias_sb[:, 3:4])
            nc.sync.dma_start(out=out_d.rearrange("(p o) -> p o", o=1), in_=o_sb)

    nc.finalize()
    return nc


def _get_nc(L: int, qidx: int):
    key = (L, qidx)
    if key not in _BUILD_CACHE:
        _BUILD_CACHE[key] = _build(L, qidx)
    return _BUILD_CACHE[key]


def _prep_in_maps(matrix, Wq, bq, Wk, bk, Wv, bv, Wo, bo, px, py):
    px = int(px)
    py = int(py)
    # replicate the reference's slice semantics exactly via index arrays
    rows = np.arange(H)[px - WIN:px + WIN + 1]
    cols = np.arange(W)[py - WIN:py + WIN + 1]
    L = len(cols)
    gr = rows[px]  # out[px, py] reads window row px -> grid row rows[px]
    qidx = py      # query token is window col py

    tokens = np.asarray(matrix[gr][cols], dtype=np.float32)      # [L, E]
    tokT = np.ascontiguousarray(tokens.T)                        # [E, L]

    in_maps = []
    for c in range(N_CORES):
        fc = slice(128 * c, 128 * (c + 1))
        in_maps.append({
            "tokT": tokT,
            "wqT": np.ascontiguousarray(Wq[fc].T),
            "wkT": np.ascontiguousarray(Wk[fc].T),
            "wvT": np.ascontiguousarray(Wv[fc].T),
            "woT": np.ascontiguousarray(Wo[fc].T),
            "biases": np.ascontiguousarray(
                np.stack([bq[fc], bk[fc], bv[fc], bo[fc]], axis=1)
            ),
        })
    return in_maps, L, qidx


def kernel(matrix, Wq, bq, Wk, bk, Wv, bv, Wo, bo, px, py, _trace=False, **_kw):
    in_maps, L, qidx = _prep_in_maps(
        matrix, Wq, bq, Wk, bk, Wv, bv, Wo, bo, px, py
    )
    nc = _get_nc(L, qidx)
    res = run_bass_kernel_spmd(
        nc, in_maps, core_ids=list(range(N_CORES)), trace=_trace
    )
    out = np.concatenate([res.results[c]["out"] for c in range(N_CORES)])
    if _trace:
        return out.astype(np.float32), res
    return out.astype(np.float32)


# revision 21
# speedup vs baseline: 2.6852x; 2.6852x over previous
"""Trainium2 Bass kernel for nn_LocalAttention_28518582845970.

The reference projects the full 256x256x1024 grid through Q/K/V/O but
returns only out[px, py] -- a single 1024-vector.  That vector depends
on one window row: 129 tokens, one query token, and the four 1024x1024
weights (by linearity, softmax shift-invariance, and sum(attn)==1):

    q      = Wq t_q + bq
    u      = Wk^T q                      (the q.bk term is constant in k
                                          -> dropped: softmax invariant)
    scores = tokens @ u
    attn   = softmax(scores/32)
    t_avg  = attn @ tokens
    out_c  = Wo_c (Wv t_avg + bv) + bo_c

v3: zero collectives (measured 25-55us each on this mesh -- they
dominate everything); every core redundantly runs the chain above and
computes only its 128-row slice of the output projection; host
concatenates.  The u-substitution means K and V are never materialized:
the whole kernel is ~100 matmuls of matvec shape.  Matmul operands are
fp16 (fp32 PE matmul is 2-pass/quarter-rate; fp16 is full rate and
halves the 12.6 MiB weight DMA), accumulation is fp32 in PSUM, softmax
and the output projection are fp32.
"""

import os
import sys

os.environ.setdefault("JAX_PLATFORMS", "axon")

for _p in ("/opt/trn_rl_repo", "/root/.axon_site/_ro/trn_rl_repo"):
    if os.path.isdir(_p) and _p not in sys.path:
        sys.path.append(_p)

import numpy as np

import concourse.bass as bass
import concourse.mybir as mybir
import concourse.tile as tile
from concourse import bacc
from concourse.bass_utils import run_bass_kernel_spmd
from concourse.masks import make_identity

N_CORES = 8
E = 1024
EC = E // 128
WIN = 64
H = W = 256
SCALE = 1.0 / 32.0
F32 = mybir.dt.float32
F16 = mybir.dt.float16

_BUILD_CACHE: dict = {}

# Lighter Tile finale: the stock _drain_and_barrier emits drain + full
# EVSEM barrier + sem clears + second barrier (~10-16us measured on this
# part).  With no collectives and per-core-independent work we keep the
# drain (output DMA completion) and sem clears behind a sem-only
# barrier, dropping the heavy drain-barrier sandwich.
from concourse.vector_clock import ScopedClock as _ScopedClock


def _light_drain_and_barrier(self, tick_clock, wait_clock):
    drain_inst = self.nc.sync.drain()
    wait_clock.add_sem_waits(
        drain_inst.ins, _ScopedClock({None: tick_clock.global_clock})
    )
    self.nc.all_engine_barrier(sem_only=True)
    popped = self.nc._tile_sem_poison_stack.pop()
    assert popped is self._sem_poison
    self.nc.clear_and_free_semaphores(list(self.sems.allocated().values()))
    self.nc.all_engine_barrier(sem_only=True)


tile.TileContext._drain_and_barrier = _light_drain_and_barrier


def _build(L: int, qidx: int):
    KA = min(128, L)          # k-chunk A: tokens [0:KA]
    BS = max(0, L - KA)       # k-chunk B start: tokens [BS:L] (overlap OK)
    nb = 3 * EC + 1           # bias columns: bq(8) bv(8) bo(1) -- packed [128, 17]

    nc = bacc.Bacc(None, target_bir_lowering=False, debug=False)

    tokT_d = nc.dram_tensor("tokT", [E, L], F16, kind="ExternalInput")
    tokN_d = nc.dram_tensor("tokN", [L, E], F16, kind="ExternalInput")
    wqT_d = nc.dram_tensor("wqT", [E, E], F16, kind="ExternalInput")   # (e, f)
    wkN_d = nc.dram_tensor("wkN", [E, E], F16, kind="ExternalInput")   # (f, e) native
    wvT_d = nc.dram_tensor("wvT", [E, E], F16, kind="ExternalInput")   # (e, f)
    woT_d = nc.dram_tensor("woT", [E, 128], F16, kind="ExternalInput")
    bias_d = nc.dram_tensor("biases", [128, 2 * EC + 1], F32, kind="ExternalInput")
    out_d = nc.dram_tensor("out", [128], F32, kind="ExternalOutput")

    wqT_r = wqT_d.rearrange("(c p) f -> p c f", p=128)
    wkN_r = wkN_d.rearrange("(c p) e -> p c e", p=128)
    wvT_r = wvT_d.rearrange("(c p) f -> p c f", p=128)
    FH = [slice(0, 512), slice(512, 1024)]

    with tile.TileContext(nc) as tc:
        with (
            tc.tile_pool(name="consts", bufs=1) as consts,
            tc.tile_pool(name="sbw", bufs=1) as sbw,
            tc.tile_pool(name="psS", bufs=2, space="PSUM") as psS,
        ):
            # ---- loads ----
            tok_sb = consts.tile([128, EC, L], F16)
            nc.sync.dma_start(out=tok_sb, in_=tokT_d.rearrange("(c p) k -> p c k", p=128))
            bias_sb = consts.tile([128, 2 * EC + 1], F32)
            nc.sync.dma_start(out=bias_sb, in_=bias_d[:, :])

            wq_sb = consts.tile([128, EC, E], F16)
            for c in range(EC):
                nc.sync.dma_start(out=wq_sb[:, c, :], in_=wqT_r[:, c, :])
            wk_sb = consts.tile([128, EC, E], F16)
            for c in range(EC):
                nc.sync.dma_start(out=wk_sb[:, c, :], in_=wkN_r[:, c, :])
            wv_sb = consts.tile([128, EC, E], F16)
            for c in range(EC):
                nc.sync.dma_start(out=wv_sb[:, c, :], in_=wvT_r[:, c, :])
            wo_sb = consts.tile([128, EC, 128], F16)
            nc.sync.dma_start(out=wo_sb, in_=woT_d.rearrange("(c p) f -> p c f", p=128))

            tokN_sb = consts.tile([128, EC, 128], F16)
            nc.sync.dma_start(
                out=tokN_sb,
                in_=tokN_d[0:KA].rearrange("k (c p) -> k c p", p=128),
            )
            if L > KA:
                tokNt_sb = consts.tile([L - KA, EC, 128], F16)
                nc.sync.dma_start(
                    out=tokNt_sb,
                    in_=tokN_d[KA:L].rearrange("k (c p) -> k c p", p=128),
                )

            ones16 = consts.tile([1, 128], F16)
            nc.vector.memset(ones16, 1.0)

            # ---- q columns: q[fc] = sum_ec WqT[ec,fc]^T @ t_q (+bq) ----
            # weights stationary ([128,128] fp16 -> fast weight load)
            q_ps = psS.tile([128, EC], F32, tag="qc", bufs=1)
            for fc in range(EC):
                fsl = slice(128 * fc, 128 * (fc + 1))
                for c in range(EC):
                    nc.tensor.matmul(
                        q_ps[:, fc:fc + 1], wq_sb[:, c, fsl],
                        tok_sb[:, c, qidx:qidx + 1],
                        start=(c == 0), stop=(c == EC - 1),
                    )
            q_cols = sbw.tile([128, EC], F16)
            nc.vector.tensor_add(q_cols, q_ps, bias_sb[:, 0:EC])

            # ---- u columns: u[ec] = sum_fc WkN[fc,ec]^T @ q_col[fc] ----
            u_ps = psS.tile([128, EC], F32, tag="uc", bufs=1)
            for ec in range(EC):
                esl = slice(128 * ec, 128 * (ec + 1))
                for c in range(EC):
                    nc.tensor.matmul(
                        u_ps[:, ec:ec + 1], wk_sb[:, c, esl], q_cols[:, c:c + 1],
                        start=(c == 0), stop=(c == EC - 1),
                    )
            u_cols = sbw.tile([128, EC], F16)
            # fold the 1/sqrt(E) score scale into u
            nc.vector.tensor_scalar_mul(u_cols, u_ps, SCALE)

            # ---- scores = u^T @ tokens -> [1, L] directly in row form ----
            s_ps = psS.tile([1, L], F32, tag="sacc", bufs=1)
            for c in range(EC):
                nc.tensor.matmul(s_ps, u_cols[:, c:c + 1], tok_sb[:, c, :],
                                 start=(c == 0), stop=(c == EC - 1))
            sc_row = sbw.tile([1, L], F32)
            nc.vector.tensor_copy(sc_row, s_ps)

            # ---- softmax (scores pre-scaled; |s| <= ~10 so no max-sub
            # needed for fp32 exp -- same result as the reference's
            # max-subtracted softmax) ----
            ex_row = sbw.tile([1, L], F32)
            sm = sbw.tile([1, 1], F32)
            nc.scalar.activation(ex_row, sc_row, mybir.ActivationFunctionType.Exp,
                                 bias=0.0, scale=1.0, accum_out=sm)
            rs = sbw.tile([1, 1], F32)
            nc.vector.reciprocal(rs, sm)
            at16 = sbw.tile([1, L], F16)
            nc.vector.tensor_scalar_mul(at16, ex_row, rs)

            # ---- t_avg = attn @ tokens on PE (tokens in [k, e] layout) ----
            atc_ps = psS.tile([128, 1], F16, tag="s")
            nc.tensor.transpose(atc_ps, at16[0:1, 0:KA], ones16[0:1, 0:1])
            at_colA = sbw.tile([KA, 1], F16)
            nc.vector.tensor_copy(at_colA, atc_ps)
            if L > KA:
                at_tail = sbw.tile([L - KA, 1], F16)
                nc.vector.tensor_copy(at_tail, at16[0:1, KA:L])
            tv_ps = psS.tile([128, EC], F32, tag="tv", bufs=1)
            for c in range(EC):
                nc.tensor.matmul(
                    tv_ps[:, c:c + 1], tokN_sb[:, c, :], at_colA,
                    start=True, stop=(L <= KA),
                )
                if L > KA:
                    nc.tensor.matmul(
                        tv_ps[:, c:c + 1], tokNt_sb[0:1, c, :], at_tail,
                        start=False, stop=True,
                    )
            tv_cols = sbw.tile([128, EC], F16)
            nc.vector.tensor_copy(tv_cols, tv_ps)

            # ---- ctx columns: ctx[fc] = sum_ec WvT[ec,fc]^T @ t_avg[ec] + bv ----
            c_ps = psS.tile([128, EC], F32, tag="cc", bufs=1)
            for fc in range(EC):
                fsl = slice(128 * fc, 128 * (fc + 1))
                for c in range(EC):
                    nc.tensor.matmul(
                        c_ps[:, fc:fc + 1], wv_sb[:, c, fsl], tv_cols[:, c:c + 1],
                        start=(c == 0), stop=(c == EC - 1),
                    )
            ctx_cols = sbw.tile([128, EC], F16)
            nc.vector.tensor_add(ctx_cols, c_ps, bias_sb[:, EC:2 * EC])

            # ---- out_c = WoT_c^T @ ctx + bo_c ----
            o_ps = psS.tile([128, 1], F32, tag="s")
            for c in range(EC):
                nc.tensor.matmul(
                    o_ps, wo_sb[:, c, :], ctx_cols[:, c:c + 1],
                    start=(c == 0), stop=(c == EC - 1),
                )
            o_sb = sbw.tile([128, 1], F32)
            nc.vector.tensor_scalar_add(o_sb, o_ps, bias_sb[:, 2 * EC:2 * EC + 1])
            nc.sync.dma_start(out=out_d.rearrange("(p o) -> p o", o=1), in_=o_sb)

    nc.finalize()
    return nc


def _get_nc(L: int, qidx: int):
    key = (L, qidx)
    if key not in _BUILD_CACHE:
        _BUILD_CACHE[key] = _build(L, qidx)
    return _BUILD_CACHE[key]


def _prep_in_maps(matrix, Wq, bq, Wk, bk, Wv, bv, Wo, bo, px, py):
    px = int(px)
    py = int(py)
    rows = np.arange(H)[px - WIN:px + WIN + 1]
    cols = np.arange(W)[py - WIN:py + WIN + 1]
    L = len(cols)
    gr = rows[px]
    qidx = py

    tokens = np.asarray(matrix[gr][cols], dtype=np.float32)        # [L, E]
    tokT = np.ascontiguousarray(tokens.T).astype(np.float16)       # [E, L]
    tokN = np.ascontiguousarray(tokens).astype(np.float16)         # [L, E]
    wqT = np.ascontiguousarray(np.asarray(Wq, np.float32).T).astype(np.float16)
    wkN = np.ascontiguousarray(np.asarray(Wk, np.float32)).astype(np.float16)
    wvT = np.ascontiguousarray(np.asarray(Wv, np.float32).T).astype(np.float16)

    bq_c = np.asarray(bq, np.float32).reshape(EC, 128).T           # [128, EC]
    bv_c = np.asarray(bv, np.float32).reshape(EC, 128).T

    in_maps = []
    for c in range(N_CORES):
        fc = slice(128 * c, 128 * (c + 1))
        biases = np.concatenate(
            [bq_c, bv_c, np.asarray(bo[fc], np.float32)[:, None]], axis=1
        )
        in_maps.append({
            "tokT": tokT,
            "tokN": tokN,
            "wqT": wqT,
            "wkN": wkN,
            "wvT": wvT,
            "woT": np.ascontiguousarray(np.asarray(Wo, np.float32)[fc].T).astype(np.float16),
            "biases": np.ascontiguousarray(biases),
        })
    return in_maps, L, qidx


def kernel(matrix, Wq, bq, Wk, bk, Wv, bv, Wo, bo, px, py, _trace=False, **_kw):
    in_maps, L, qidx = _prep_in_maps(
        matrix, Wq, bq, Wk, bk, Wv, bv, Wo, bo, px, py
    )
    nc = _get_nc(L, qidx)
    res = run_bass_kernel_spmd(
        nc, in_maps, core_ids=list(range(N_CORES)), trace=_trace
    )
    out = np.concatenate([res.results[c]["out"] for c in range(N_CORES)])
    if _trace:
        return out.astype(np.float32), res
    return out.astype(np.float32)


# revision 23
# speedup vs baseline: 2.8120x; 1.0472x over previous
"""Trainium2 Bass kernel for nn_LocalAttention_28518582845970.

The reference projects the full 256x256x1024 grid through Q/K/V/O but
returns only out[px, py] -- a single 1024-vector.  That vector depends
on one window row: 129 tokens, one query token, and the four 1024x1024
weights (by linearity, softmax shift-invariance, and sum(attn)==1):

    q      = Wq t_q + bq
    u      = Wk^T q                      (the q.bk term is constant in k
                                          -> dropped: softmax invariant)
    scores = tokens @ u
    attn   = softmax(scores/32)
    t_avg  = attn @ tokens
    out_c  = Wo_c (Wv t_avg + bv) + bo_c

v3: zero collectives (measured 25-55us each on this mesh -- they
dominate everything); every core redundantly runs the chain above and
computes only its 128-row slice of the output projection; host
concatenates.  The u-substitution means K and V are never materialized:
the whole kernel is ~100 matmuls of matvec shape.  Matmul operands are
fp16 (fp32 PE matmul is 2-pass/quarter-rate; fp16 is full rate and
halves the 12.6 MiB weight DMA), accumulation is fp32 in PSUM, softmax
and the output projection are fp32.
"""

import os
import sys

os.environ.setdefault("JAX_PLATFORMS", "axon,cpu")

for _p in ("/opt/trn_rl_repo", "/root/.axon_site/_ro/trn_rl_repo"):
    if os.path.isdir(_p) and _p not in sys.path:
        sys.path.append(_p)

import numpy as np

import concourse.bass as bass
import concourse.mybir as mybir
import concourse.tile as tile
from concourse import bacc
from concourse.bass_utils import run_bass_kernel_spmd
from concourse.masks import make_identity

N_CORES = 8
E = 1024
EC = E // 128
WIN = 64
H = W = 256
SCALE = 1.0 / 32.0
F32 = mybir.dt.float32
F16 = mybir.dt.float16

_BUILD_CACHE: dict = {}

# Lighter Tile finale: the stock _drain_and_barrier emits drain + full
# EVSEM barrier + sem clears + second barrier (~10-16us measured on this
# part).  With no collectives and per-core-independent work we keep the
# drain (output DMA completion) and sem clears behind a sem-only
# barrier, dropping the heavy drain-barrier sandwich.
from concourse.vector_clock import ScopedClock as _ScopedClock


def _light_drain_and_barrier(self, tick_clock, wait_clock):
    drain_inst = self.nc.sync.drain()
    wait_clock.add_sem_waits(
        drain_inst.ins, _ScopedClock({None: tick_clock.global_clock})
    )
    self.nc.all_engine_barrier(sem_only=True)
    popped = self.nc._tile_sem_poison_stack.pop()
    assert popped is self._sem_poison
    self.nc.clear_and_free_semaphores(list(self.sems.allocated().values()))
    self.nc.all_engine_barrier(sem_only=True)


tile.TileContext._drain_and_barrier = _light_drain_and_barrier


def _build(L: int, qidx: int):
    KA = min(128, L)          # k-chunk A: tokens [0:KA]
    BS = max(0, L - KA)       # k-chunk B start: tokens [BS:L] (overlap OK)
    nb = 3 * EC + 1           # bias columns: bq(8) bv(8) bo(1) -- packed [128, 17]

    nc = bacc.Bacc(None, target_bir_lowering=False, debug=False)

    tokT_d = nc.dram_tensor("tokT", [E, L], F16, kind="ExternalInput")
    tokN_d = nc.dram_tensor("tokN", [L, E], F16, kind="ExternalInput")
    wqT_d = nc.dram_tensor("wqT", [E, E], F16, kind="ExternalInput")   # (e, f)
    wkN_d = nc.dram_tensor("wkN", [E, E], F16, kind="ExternalInput")   # (f, e) native
    wvT_d = nc.dram_tensor("wvT", [E, E], F16, kind="ExternalInput")   # (e, f)
    woT_d = nc.dram_tensor("woT", [E, 128], F16, kind="ExternalInput")
    bias_d = nc.dram_tensor("biases", [128, 2 * EC + 1], F32, kind="ExternalInput")
    out_d = nc.dram_tensor("out", [128], F32, kind="ExternalOutput")

    wqT_r = wqT_d.rearrange("(c p) f -> p c f", p=128)
    wkN_r = wkN_d.rearrange("(c p) e -> p c e", p=128)
    wvT_r = wvT_d.rearrange("(c p) f -> p c f", p=128)
    FH = [slice(0, 512), slice(512, 1024)]

    with tile.TileContext(nc) as tc:
        with (
            tc.tile_pool(name="consts", bufs=1) as consts,
            tc.tile_pool(name="sbw", bufs=1) as sbw,
            tc.tile_pool(name="psS", bufs=2, space="PSUM") as psS,
        ):
            # ---- loads ----
            tok_sb = consts.tile([128, EC, L], F16)
            nc.sync.dma_start(out=tok_sb, in_=tokT_d.rearrange("(c p) k -> p c k", p=128))
            bias_sb = consts.tile([128, 2 * EC + 1], F32)
            nc.sync.dma_start(out=bias_sb, in_=bias_d[:, :])

            wq_sb = consts.tile([128, EC, E], F16)
            for c in range(EC):
                nc.sync.dma_start(out=wq_sb[:, c, :], in_=wqT_r[:, c, :])
            wk_sb = consts.tile([128, EC, E], F16)
            for c in range(EC):
                nc.sync.dma_start(out=wk_sb[:, c, :], in_=wkN_r[:, c, :])
            wv_sb = consts.tile([128, EC, E], F16)
            for c in range(EC):
                nc.sync.dma_start(out=wv_sb[:, c, :], in_=wvT_r[:, c, :])
            wo_sb = consts.tile([128, EC, 128], F16)
            nc.sync.dma_start(out=wo_sb, in_=woT_d.rearrange("(c p) f -> p c f", p=128))

            tokN_sb = consts.tile([128, EC, 128], F16)
            nc.sync.dma_start(
                out=tokN_sb,
                in_=tokN_d[0:KA].rearrange("k (c p) -> k c p", p=128),
            )
            if L > KA:
                tokNt_sb = consts.tile([L - KA, EC, 128], F16)
                nc.sync.dma_start(
                    out=tokNt_sb,
                    in_=tokN_d[KA:L].rearrange("k (c p) -> k c p", p=128),
                )

            ones16 = consts.tile([1, 128], F16)
            nc.vector.memset(ones16, 1.0)
            warm16 = consts.tile([128, 128], F16)
            nc.vector.memset(warm16, 0.0)

            # PE-HAM warmup: sustained dummy matmuls while weights stream in,
            # so the real chain runs at the unthrottled clock.
            wu_ps = psS.tile([128, 1], F32, tag="wu", bufs=1)
            for w in range(100):
                nc.tensor.matmul(wu_ps, warm16, warm16[:, 0:1],
                                 start=(w == 0), stop=(w == 99))

            # ---- q columns: q[fc] = sum_ec WqT[ec,fc]^T @ t_q (+bq) ----
            # weights stationary ([128,128] fp16 -> fast weight load)
            q_ps = psS.tile([128, EC], F32, tag="qc", bufs=1)
            for fc in range(EC):
                fsl = slice(128 * fc, 128 * (fc + 1))
                for c in range(EC):
                    nc.tensor.matmul(
                        q_ps[:, fc:fc + 1], wq_sb[:, c, fsl],
                        tok_sb[:, c, qidx:qidx + 1],
                        start=(c == 0), stop=(c == EC - 1),
                    )
            q_cols = sbw.tile([128, EC], F16)
            nc.vector.tensor_add(q_cols, q_ps, bias_sb[:, 0:EC])

            # ---- u columns: u[ec] = sum_fc WkN[fc,ec]^T @ q_col[fc] ----
            u_ps = psS.tile([128, EC], F32, tag="uc", bufs=1)
            for ec in range(EC):
                esl = slice(128 * ec, 128 * (ec + 1))
                for c in range(EC):
                    nc.tensor.matmul(
                        u_ps[:, ec:ec + 1], wk_sb[:, c, esl], q_cols[:, c:c + 1],
                        start=(c == 0), stop=(c == EC - 1),
                    )
            u_cols = sbw.tile([128, EC], F16)
            # fold the 1/sqrt(E) score scale into u
            nc.vector.tensor_scalar_mul(u_cols, u_ps, SCALE)

            # ---- scores = u^T @ tokens -> [1, L] directly in row form ----
            s_ps = psS.tile([1, L], F32, tag="sacc", bufs=1)
            for c in range(EC):
                nc.tensor.matmul(s_ps, u_cols[:, c:c + 1], tok_sb[:, c, :],
                                 start=(c == 0), stop=(c == EC - 1))

            # ---- softmax (scores pre-scaled; |s| <= ~10 so no max-sub
            # needed for fp32 exp -- same result as the reference's
            # max-subtracted softmax) ----
            ex_row = sbw.tile([1, L], F32)
            sm = sbw.tile([1, 1], F32)
            nc.scalar.activation(ex_row, s_ps, mybir.ActivationFunctionType.Exp,
                                 bias=0.0, scale=1.0, accum_out=sm)
            rs = sbw.tile([1, 1], F32)
            nc.vector.reciprocal(rs, sm)
            at16 = sbw.tile([1, L], F16)
            nc.vector.tensor_scalar_mul(at16, ex_row, rs)

            # ---- t_avg = attn @ tokens on PE (tokens in [k, e] layout) ----
            atc_ps = psS.tile([128, 1], F16, tag="s")
            nc.tensor.transpose(atc_ps, at16[0:1, 0:KA], ones16[0:1, 0:1])
            at_colA = sbw.tile([KA, 1], F16)
            nc.vector.tensor_copy(at_colA, atc_ps)
            if L > KA:
                at_tail = sbw.tile([L - KA, 1], F16)
                nc.vector.tensor_copy(at_tail, at16[0:1, KA:L])
            tv_ps = psS.tile([128, EC], F32, tag="tv", bufs=1)
            for c in range(EC):
                nc.tensor.matmul(
                    tv_ps[:, c:c + 1], tokN_sb[:, c, :], at_colA,
                    start=True, stop=(L <= KA),
                )
                if L > KA:
                    nc.tensor.matmul(
                        tv_ps[:, c:c + 1], tokNt_sb[0:1, c, :], at_tail,
                        start=False, stop=True,
                    )
            tv_cols = sbw.tile([128, EC], F16)
            nc.vector.tensor_copy(tv_cols, tv_ps)

            # ---- ctx columns: ctx[fc] = sum_ec WvT[ec,fc]^T @ t_avg[ec] + bv ----
            c_ps = psS.tile([128, EC], F32, tag="cc", bufs=1)
            for fc in range(EC):
                fsl = slice(128 * fc, 128 * (fc + 1))
                for c in range(EC):
                    nc.tensor.matmul(
                        c_ps[:, fc:fc + 1], wv_sb[:, c, fsl], tv_cols[:, c:c + 1],
                        start=(c == 0), stop=(c == EC - 1),
                    )
            ctx_cols = sbw.tile([128, EC], F16)
            nc.vector.tensor_add(ctx_cols, c_ps, bias_sb[:, EC:2 * EC])

            # ---- out_c = WoT_c^T @ ctx + bo_c ----
            o_ps = psS.tile([128, 1], F32, tag="s")
            for c in range(EC):
                nc.tensor.matmul(
                    o_ps, wo_sb[:, c, :], ctx_cols[:, c:c + 1],
                    start=(c == 0), stop=(c == EC - 1),
                )
            o_sb = sbw.tile([128, 1], F32)
            nc.vector.tensor_scalar_add(o_sb, o_ps, bias_sb[:, 2 * EC:2 * EC + 1])
            nc.sync.dma_start(out=out_d.rearrange("(p o) -> p o", o=1), in_=o_sb)

    nc.finalize()
    return nc


def _get_nc(L: int, qidx: int):
    key = (L, qidx)
    if key not in _BUILD_CACHE:
        _BUILD_CACHE[key] = _build(L, qidx)
    return _BUILD_CACHE[key]


def _prep_in_maps(matrix, Wq, bq, Wk, bk, Wv, bv, Wo, bo, px, py):
    px = int(px)
    py = int(py)
    rows = np.arange(H)[px - WIN:px + WIN + 1]
    cols = np.arange(W)[py - WIN:py + WIN + 1]
    L = len(cols)
    gr = rows[px]
    qidx = py

    tokens = np.asarray(matrix[gr][cols], dtype=np.float32)        # [L, E]
    tokT = np.ascontiguousarray(tokens.T).astype(np.float16)       # [E, L]
    tokN = np.ascontiguousarray(tokens).astype(np.float16)         # [L, E]
    wqT = np.ascontiguousarray(np.asarray(Wq, np.float32).T).astype(np.float16)
    wkN = np.ascontiguousarray(np.asarray(Wk, np.float32)).astype(np.float16)
    wvT = np.ascontiguousarray(np.asarray(Wv, np.float32).T).astype(np.float16)

    bq_c = np.asarray(bq, np.float32).reshape(EC, 128).T           # [128, EC]
    bv_c = np.asarray(bv, np.float32).reshape(EC, 128).T

    in_maps = []
    for c in range(N_CORES):
        fc = slice(128 * c, 128 * (c + 1))
        biases = np.concatenate(
            [bq_c, bv_c, np.asarray(bo[fc], np.float32)[:, None]], axis=1
        )
        in_maps.append({
            "tokT": tokT,
            "tokN": tokN,
            "wqT": wqT,
            "wkN": wkN,
            "wvT": wvT,
            "woT": np.ascontiguousarray(np.asarray(Wo, np.float32)[fc].T).astype(np.float16),
            "biases": np.ascontiguousarray(biases),
        })
    return in_maps, L, qidx


def kernel(matrix, Wq, bq, Wk, bk, Wv, bv, Wo, bo, px, py, _trace=False, **_kw):
    in_maps, L, qidx = _prep_in_maps(
        matrix, Wq, bq, Wk, bk, Wv, bv, Wo, bo, px, py
    )
    nc = _get_nc(L, qidx)
    res = run_bass_kernel_spmd(
        nc, in_maps, core_ids=list(range(N_CORES)), trace=_trace
    )
    out = np.concatenate([res.results[c]["out"] for c in range(N_CORES)])
    if _trace:
        return out.astype(np.float32), res
    return out.astype(np.float32)
